# revision 1
# baseline (speedup 1.0000x reference)
"""Trainium2 Bass kernel for GroupedVectorSA (gnn message passing), v2.

Self-contained: accepts FULL inputs (as produced by setup_inputs()), shards
across 8 NeuronCores (batch b = core//4, quarter of N = core%4), runs one
SPMD Bass/Tile program via bass_utils.run_bass_kernel_spmd, reassembles the
full [B, N, C] output.

v2 design notes (vs v1 baseline ~700us):
  - All K=256 projections run as fp8e4(e4m3) DoubleRow matmuls (0.5 cyc/row).
  - Host pre-transposes every operand (no on-device DMA transposes) and
    pre-gathers neighbor feature rows (input-only work).
  - Linear biases enter PSUM via K=1 ones-row matmuls; BN affines fold into
    weights host-side; per-matrix power-of-2 scales keep fp8 operands in
    range and are exactly compensated downstream.
  - relu(kg')-q fused into one DVE scalar_tensor_tensor from PSUM.
  - val = vg + peb accumulated inside PSUM by the PE (no vector add).
  - softmax: unnormalized e drives the weighted sum; the denominator
    (esum) ships to HBM and the host divides during the unshard, along
    with the constant output bias (pb_b2 + bv).
  - PSUM choreography: one rotating 3-buffer [128,1024] tag for all
    short-lived psum tensors + 1 buffer for val (lives across the softmax);
    prod/outp run software-pipelined one tile behind so the in-order DVE
    queue (t1, t2, prod) never stalls.
  - Engine balance: Scalar h1 acts + hw/exp; DVE t1/t2/prod; Pool esum +
    S-window output reduce.
"""

import os
import sys

import numpy as np

try:
    import concourse  # noqa: F401
except ImportError:
    sys.path.insert(0, "/opt/trn_rl_repo")

import ml_dtypes

import concourse.bacc as bacc
import concourse.bass as bass  # noqa: F401
import concourse.mybir as mybir
import concourse.tile as tile
from concourse import bass_utils

F32 = mybir.dt.float32
BF16 = mybir.dt.bfloat16
FP16 = mybir.dt.float16
FP8 = mybir.dt.float8e4

NP_BF16 = ml_dtypes.bfloat16
NP_FP8 = ml_dtypes.float8_e4m3fn

EPS = 1e-5
B, N, S, C, G = 2, 4096, 16, 256, 8
NCORES = 8
CPB = NCORES // B          # cores per batch = 4
NLOC = N // CPB            # points per core = 1024
NPT = 32                   # points per compute tile
RT = NPT * S               # gathered rows per compute tile = 512
NTILES = NLOC // NPT       # 32
NCHUNK = 4                 # fgT8 DMA chunks
TPC = NTILES // NCHUNK     # tiles per chunk = 8
CCOLS = NLOC * S // NCHUNK  # columns per chunk = 4096

# power-of-2 fp8 range scales (exactly compensated downstream)
S_QW = 8.0     # wq
S_KW = 4.0     # wk_eff; kg-psum, t1, qm carry S_KW
S_W2M = 1.0    # pm_w2 now bf16; t2 carries S_KW
S_WE1 = 1.0    # we_w1 bf16
S_FW = S_KW    # F bf16 must match t2's S_KW
S_VAL = 8.0    # wv and pb_w2 (val-psum, outacc carry S_VAL)
S_W1 = 4.0     # pm_w1/pb_w1 fp8 lhsT scale (h1 act divides back)

AO = mybir.AluOpType
AF = mybir.ActivationFunctionType
AX = mybir.AxisListType
DR = mybir.MatmulPerfMode.DoubleRow


def _affine(bn_p, lin_b):
    """Fold eval-mode BN (+ preceding linear bias) into scale/bias vectors."""
    bn_p = np.asarray(bn_p, np.float32)
    g, beta, m, v = bn_p[0], bn_p[1], bn_p[2], bn_p[3]
    s = g / np.sqrt(v + EPS)
    t = (np.asarray(lin_b, np.float32) - m) * s + beta
    return s.astype(np.float32), t.astype(np.float32)


def _as_lhst(w):
    """[256, X] -> [128, 2, X] (partition p, k-half j: k = j*128 + p)."""
    w = np.asarray(w, np.float32)
    return np.ascontiguousarray(w.reshape(2, 128, w.shape[1]).transpose(1, 0, 2))


def _per_part(vec):
    """[256] -> [128, 2]  (channel = j*128 + p)."""
    return np.ascontiguousarray(np.asarray(vec, np.float32).reshape(2, 128).T)


def build_program(h1_fp8=True):
    nc = bacc.Bacc("TRN2", target_bir_lowering=False, debug=False,
                   num_devices=NCORES)

    def din(name, shape, dt):
        return nc.dram_tensor(name, list(shape), dt, kind="ExternalInput")

    featsT8 = din("featsT8", [128, 2, NLOC], FP8)
    fgb8 = din("fgb8", [128, 2, NLOC * S], FP8)
    if h1_fp8:
        pos_d = din("pos8", [2, 2, NLOC * S], FP8)
    else:
        pos_d = din("pos4", [4, NLOC * S], BF16)
    consts = [
        ("wq8", [128, 2, C], FP8), ("wke8", [128, 2, C], FP8),
        ("wv8", [128, 2, C], FP8),
        ("w2m", [128, 2, C], BF16), ("w2b", [128, 2, C], BF16),
        ("we1", [128, 2, 2 * G], BF16), ("fw", [128, 2, 2 * G], BF16),
        ("sqv", [128, 2], F32), ("tqv", [128, 2], F32),
        ("tkv", [128, 2], F32), ("b2v", [128, 2], F32),
        ("swe", [G, 1], F32), ("twe", [G, 1], F32),
        ("we2", [G, G], BF16), ("web2", [G, 1], F32),
        ("eoh", [G, 2, 128], BF16),
    ]
    if h1_fp8:
        consts += [("w1m8", [2, 2, C], FP8), ("w1b8", [2, 2, C], FP8)]
    else:
        consts += [("w1m", [4, C], BF16), ("w1b", [4, C], BF16)]
    cdram = {name: din(name, shape, dt) for name, shape, dt in consts}

    out_d = nc.dram_tensor("out", [128, NTILES, 2, NPT], F32,
                           kind="ExternalOutput")
    esum_d = nc.dram_tensor("esum", [G, NLOC], F32, kind="ExternalOutput")

    with tile.TileContext(nc) as tc:
        with (
            tc.tile_pool(name="const", bufs=1) as cpool,
            tc.tile_pool(name="big", bufs=1) as bigpool,
            tc.tile_pool(name="work", bufs=2) as wpool,
            tc.tile_pool(name="small", bufs=3) as spool,
            tc.tile_pool(name="ps", bufs=2, space="PSUM") as psP,
            tc.tile_pool(name="psm", bufs=1, space="PSUM") as psM,
        ):
            csb = {}
            first = ["wq8", "sqv", "tqv"]
            for name, shape, dt in consts:
                csb[name] = cpool.tile(list(shape), dt, tag=name, name=name)
            featsT = cpool.tile([128, 2, NLOC], FP8, tag="featsT")
            for name in first:
                nc.sync.dma_start(csb[name][:], cdram[name][:])
            nc.sync.dma_start(featsT[:], featsT8[:])
            posT = cpool.tile(
                [2, 2, NLOC * S] if h1_fp8 else [4, NLOC * S],
                FP8 if h1_fp8 else BF16, tag="posT")
            nc.sync.dma_start(posT[:], pos_d[:])
            for name, shape, dt in consts:
                if name not in first:
                    nc.sync.dma_start(csb[name][:], cdram[name][:])
            fgc = []
            for cch in range(NCHUNK):
                t = cpool.tile([128, 2, CCOLS], FP8, tag=f"fg{cch}")
                nc.sync.dma_start(t[:], fgb8[:, :, cch * CCOLS:(cch + 1) * CCOLS])
                fgc.append(t)

            # tile-local layout [p, tile, j, n] so (j, n) flattens contiguous
            outacc = bigpool.tile([128, NTILES, 2, NPT], F32, tag="outacc")
            esumacc = bigpool.tile([G, NLOC], F32, tag="esumacc")

            # ---- q phase: qm = S_KW * relu(bn_q(feats @ wq + bq)) ----------
            # tile-local layout [p, tile, j, n] so (j, n) flattens contiguous
            qm = bigpool.tile([128, NTILES, 2, NPT], BF16, tag="qm")
            tpch = 512 // NPT  # tiles per 512-point chunk
            for ch in range(NLOC // 512):
                pq = psP.tile([128, 1024], F32, tag="rot")
                for mj in range(2):
                    nc.tensor.matmul(
                        pq[:, mj * 512:(mj + 1) * 512],
                        csb["wq8"][:, :, mj * 128:(mj + 1) * 128],
                        featsT[:, :, ch * 512:(ch + 1) * 512],
                        start=True, stop=True, perf_mode=DR)
                for mj in range(2):
                    nc.scalar.activation(
                        qm[:, ch * tpch:(ch + 1) * tpch, mj, :],
                        pq[:, mj * 512:(mj + 1) * 512]
                            .rearrange("p (t n) -> p t n", n=NPT),
                        AF.Relu,
                        bias=csb["tqv"][:, mj:mj + 1],
                        scale=csb["sqv"][:, mj:mj + 1])

            # ---- main tile loop ------------------------------------------
            # h1 stage runs one tile AHEAD; prod/outp one tile BEHIND.
            prev = None  # (pv, ebb, ti) of previous tile

            def emit_h1(t):
                g0 = t * RT
                h18 = wpool.tile([128, 2, 2, 512], BF16, tag="h18")
                for mj in range(2):
                    ph = psP.tile([128, 1024], F32, tag="rot")
                    for mlp, wkey in ((0, "m"), (1, "b")):
                        if h1_fp8:
                            nc.tensor.matmul(
                                ph[:, mlp * 512:(mlp + 1) * 512],
                                csb[f"w1{wkey}8"][:, :, mj * 128:(mj + 1) * 128],
                                posT[:, :, g0:g0 + RT],
                                start=True, stop=True, perf_mode=DR)
                        else:
                            nc.tensor.matmul(
                                ph[:, mlp * 512:(mlp + 1) * 512],
                                csb[f"w1{wkey}"][:, mj * 128:(mj + 1) * 128],
                                posT[:, g0:g0 + RT],
                                start=True, stop=True)
                    nc.scalar.activation(
                        h18[:, :, mj, :], ph[:].rearrange("p (l n) -> p l n", l=2),
                        AF.Relu, scale=1.0 / S_W1 if h1_fp8 else 1.0)
                return h18

            def emit_prod(valb, ebb, ti):
                # prod = val'(SBUF) * ebb(SBUF)  on DVE
                prod = wpool.tile([128, 64, S], BF16, tag="prod")
                nc.vector.scalar_tensor_tensor(
                    prod[:], valb[:].rearrange("p (m s) -> p m s", s=S),
                    0.0, ebb[:].rearrange("p (m s) -> p m s", s=S),
                    op0=AO.add, op1=AO.mult)
                # S-window reduce: pool halving adds, then tiny DVE reduce
                p8 = wpool.tile([128, 64, S // 2], F32, tag="p8")
                nc.gpsimd.tensor_add(p8[:], prod[:, :, 0:8], prod[:, :, 8:16])
                p4 = wpool.tile([128, 64, S // 4], F32, tag="p4")
                nc.gpsimd.tensor_add(p4[:], p8[:, :, 0:4], p8[:, :, 4:8])
                p2 = wpool.tile([128, 64, S // 8], F32, tag="p2")
                nc.gpsimd.tensor_add(p2[:], p4[:, :, 0:2], p4[:, :, 2:4])
                nc.vector.reduce_sum(
                    outacc[:, ti, :, :].rearrange("p j n -> p (j n)"),
                    p2[:], axis=AX.X)

            h18_cur = emit_h1(0)

            for t in range(NTILES):
                pt0 = t * NPT
                fg = fgc[t // TPC]
                r0 = (t % TPC) * RT

                # pos-path hidden layers were computed one tile ahead
                h18 = h18_cur
                h1m8 = h18[:, 0, :, :]
                h1b8 = h18[:, 1, :, :]

                # kg' = S_KW*(wk_eff @ fg + tk); t1 = relu(kg') - qm
                pk = psP.tile([128, 1024], F32, tag="rot")
                for mj in range(2):
                    nc.tensor.matmul(
                        pk[:, mj * 512:(mj + 1) * 512],
                        csb["wke8"][:, :, mj * 128:(mj + 1) * 128],
                        fg[:, :, r0:r0 + RT],
                        start=True, stop=True, perf_mode=DR)
                kgt = wpool.tile([128, 2, RT], BF16, tag="kgt")
                for mj in range(2):
                    nc.scalar.activation(
                        kgt[:, mj, :], pk[:, mj * 512:(mj + 1) * 512],
                        AF.Relu, bias=csb["tkv"][:, mj:mj + 1])
                t1 = wpool.tile([128, 64, S], BF16, tag="t1")
                qb = qm[:, t, :, :].rearrange("p j n -> p (j n)") \
                    .unsqueeze(2).broadcast_to((128, 64, S))
                nc.vector.scalar_tensor_tensor(
                    t1[:], kgt[:].rearrange("p j (n s) -> p (j n) s", s=S),
                    0.0, qb, op0=AO.add, op1=AO.subtract)

                # pem' = S_W2M*(pem + b2m); t2 = pem' * t1 (carries 32x)
                pp = psP.tile([128, 1024], F32, tag="rot")
                for mj in range(2):
                    for kt in range(2):
                        nc.tensor.matmul(
                            pp[:, mj * 512:(mj + 1) * 512],
                            csb["w2m"][:, kt, mj * 128:(mj + 1) * 128],
                            h1m8[:, kt, :],
                            start=(kt == 0), stop=(kt == 1))
                pemb = wpool.tile([128, 2, RT], BF16, tag="pemb")
                for mj in range(2):
                    nc.scalar.activation(
                        pemb[:, mj, :], pp[:, mj * 512:(mj + 1) * 512],
                        AF.Identity, bias=csb["b2v"][:, mj:mj + 1])
                t28 = wpool.tile([128, 2, 512], BF16, tag="t28")
                nc.vector.scalar_tensor_tensor(
                    t28[:], pemb[:],
                    0.0, t1[:].rearrange("p m s -> p (m s)")
                        .rearrange("p (j n) -> p j n", j=2),
                    op0=AO.add, op1=AO.mult)

                # val' = S_VAL*(vg + peb0): both accumulated in PSUM
                pv = psP.tile([128, 1024], F32, tag="pv", bufs=1)
                for mj in range(2):
                    nc.tensor.matmul(
                        pv[:, mj * 512:(mj + 1) * 512],
                        csb["wv8"][:, :, mj * 128:(mj + 1) * 128],
                        fg[:, :, r0:r0 + RT],
                        start=True, stop=False, perf_mode=DR)
                    for kt in range(2):
                        nc.tensor.matmul(
                            pv[:, mj * 512:(mj + 1) * 512],
                            csb["w2b"][:, kt, mj * 128:(mj + 1) * 128],
                            h1b8[:, kt, :],
                            start=False, stop=(kt == 1))
                valb = wpool.tile([128, 1024], BF16, tag="valb")
                nc.vector.tensor_scalar_mul(valb[:], pv[:], 1.0)

                # logits: lg' = 256*lg = we18^T t28 + fw8^T h1b8
                pl = psM.tile([2 * G, RT], F32, tag="misc",
                              padded_shape=[128, 1024])
                for kt in range(2):
                    nc.tensor.matmul(pl[:], csb["we1"][:, kt, :], t28[:, kt, :],
                                     start=(kt == 0), stop=False)
                for kt in range(2):
                    nc.tensor.matmul(pl[:], csb["fw"][:, kt, :], h1b8[:, kt, :],
                                     start=False, stop=(kt == 1))
                hw = spool.tile([G, RT], BF16, tag="hw")
                nc.scalar.activation(hw[:], pl[0:G, :], AF.Relu,
                                     bias=csb["twe"][:], scale=csb["swe"][:])
                pl2 = psM.tile([G, RT], F32, tag="misc",
                               padded_shape=[128, 1024])
                nc.tensor.matmul(pl2[:], csb["we2"][:], hw[:],
                                 start=True, stop=True)
                e = spool.tile([G, RT], BF16, tag="e")
                nc.scalar.activation(e[:], pl2[:], AF.Exp,
                                     bias=csb["web2"][:], scale=1.0)
                e8 = spool.tile([G, NPT, S // 2], F32, tag="e8")
                ev = e[:].rearrange("p (n s) -> p n s", s=S)
                nc.gpsimd.tensor_add(e8[:], ev[:, :, 0:8], ev[:, :, 8:16])
                e4 = spool.tile([G, NPT, S // 4], F32, tag="e4")
                nc.gpsimd.tensor_add(e4[:], e8[:, :, 0:4], e8[:, :, 4:8])
                nc.vector.reduce_sum(
                    esumacc[:, pt0:pt0 + NPT], e4[:], axis=AX.X)

                # expand e over channel groups; evacuate to SBUF on scalar
                pe = psM.tile([128, 1024], F32, tag="misc")
                for mj in range(2):
                    nc.tensor.matmul(
                        pe[:, mj * 512:(mj + 1) * 512],
                        csb["eoh"][:, mj, :], e[:],
                        start=True, stop=True)
                ebb = wpool.tile([128, 1024], BF16, tag="ebb")
                nc.vector.tensor_scalar_mul(ebb[:], pe[:], 1.0)

                # emit next tile's h1 stage (keeps PE dense, acts early)
                if t + 1 < NTILES:
                    h18_cur = emit_h1(t + 1)

                # previous tile's prod/outp (keeps DVE queue stall-free)
                if prev is not None:
                    emit_prod(*prev)
                prev = (valb, ebb, t)

            emit_prod(*prev)

            nc.sync.dma_start(out_d[:], outacc[:])
            nc.sync.dma_start(esum_d[:], esumacc[:])

    nc.compile()
    return nc


def host_prep(inputs, h1_fp8=True):
    """Fold BN, scale/cast weights to fp8, build per-core input maps."""
    f = {k: np.asarray(v) for k, v in inputs.items()}
    feats, coords, index = f["feats"], f["coords"], f["index"]
    index = index.astype(np.int64)

    s_q, t_q = _affine(f["bnq"], f["bq"])
    s_k, t_k = _affine(f["bnk"], f["bk"])
    s_hm, t_hm = _affine(f["pm_bn"], f["pm_b1"])
    s_hb, t_hb = _affine(f["pb_bn"], f["pb_b1"])

    b2b_we = np.asarray(f["pb_b2"], np.float32) @ np.asarray(f["we_w1"], np.float32)
    s_we, t_we = _affine(f["we_bn"], np.asarray(f["we_b1"], np.float32) + b2b_we)

    wk_eff = np.asarray(f["wk"], np.float32) * s_k[None, :]
    F_mat = np.asarray(f["pb_w2"], np.float32) @ np.asarray(f["we_w1"], np.float32)

    # w1' = w1 * bn_scale with ones-row bias fold (pos row 3 == 1)
    def w1_fold(w1, s_h, t_h):
        w = np.asarray(w1, np.float32) * s_h[None, :]
        return np.concatenate([w, t_h[None, :]], 0)  # [4, C]

    w1m_f = w1_fold(f["pm_w1"], s_hm, t_hm)
    w1b_f = w1_fold(f["pb_w1"], s_hb, t_hb)

    eoh = np.zeros((G, 2, 128), np.float32)
    for g in range(G):
        j, p0 = divmod(g * 32, 128)
        eoh[g, j, p0:p0 + 32] = 1.0

    shared = {
        "wq8": (_as_lhst(f["wq"]) * S_QW).astype(NP_FP8),
        "wke8": (_as_lhst(wk_eff) * S_KW).astype(NP_FP8),
        "wv8": (_as_lhst(f["wv"]) * S_VAL).astype(NP_FP8),
        "w2m": (_as_lhst(f["pm_w2"]) * S_W2M).astype(NP_BF16),
        "w2b": (_as_lhst(f["pb_w2"]) * S_VAL).astype(NP_BF16),
        "we1": np.concatenate(
            [(_as_lhst(f["we_w1"]) * S_WE1), np.zeros((128, 2, G), np.float32)],
            axis=2).astype(NP_BF16),
        "fw": np.concatenate(
            [(_as_lhst(F_mat) * S_FW), np.zeros((128, 2, G), np.float32)],
            axis=2).astype(NP_BF16),
        "sqv": _per_part(s_q * S_KW / S_QW),
        "tqv": _per_part(t_q * S_KW),
        "tkv": _per_part(t_k * S_KW),
        "b2v": _per_part(np.asarray(f["pm_b2"], np.float32) * S_W2M),
        "swe": (s_we / (S_WE1 * S_KW * S_W2M)).reshape(G, 1).astype(np.float32),
        "twe": t_we.reshape(G, 1).astype(np.float32),
        "we2": np.asarray(f["we_w2"], np.float32).astype(NP_BF16),
        "web2": np.asarray(f["we_b2"], np.float32).reshape(G, 1),
        "eoh": eoh.astype(NP_BF16),
    }
    if h1_fp8:
        # k = i*2 + p mapping for [2, 2, C] lhsT / [2, 2, cols] rhs
        def pack22(w4):  # [4, C] -> [2, 2, C]
            return np.ascontiguousarray(
                w4.reshape(2, 2, -1).transpose(1, 0, 2))
        shared["w1m8"] = (pack22(w1m_f) * S_W1).astype(NP_FP8)
        shared["w1b8"] = (pack22(w1b_f) * S_W1).astype(NP_FP8)
    else:
        shared["w1m"] = w1m_f.astype(NP_BF16)
        shared["w1b"] = w1b_f.astype(NP_BF16)

    in_maps = []
    for core in range(NCORES):
        b, qc = divmod(core, CPB)
        qoff = qc * NLOC
        fb32 = np.asarray(feats[b], np.float32)
        # featsT8: [128, 2, NLOC], [p, j, n] = feats[n, j*128+p]
        fq = fb32[qoff:qoff + NLOC]
        featsT = np.ascontiguousarray(
            fq.T.reshape(2, 128, NLOC).transpose(1, 0, 2)).astype(NP_FP8)
        idx = index[b, qoff:qoff + NLOC, :].reshape(-1)
        fg = fb32[idx]                                   # [NLOC*S, C]
        fgb8 = np.ascontiguousarray(
            fg.T.reshape(2, 128, NLOC * S).transpose(1, 0, 2)).astype(NP_FP8)
        cb = np.asarray(coords[b], np.float32)
        pos = cb[qoff:qoff + NLOC][:, None, :] - cb[idx.reshape(NLOC, S)]
        pos4 = np.concatenate(
            [pos.reshape(NLOC * S, 3).T,
             np.ones((1, NLOC * S), np.float32)], 0)     # [4, NLOC*S]
        m = dict(shared)
        m["featsT8"] = featsT
        m["fgb8"] = fgb8
        if h1_fp8:
            m["pos8"] = np.ascontiguousarray(
                pos4.reshape(2, 2, NLOC * S).transpose(1, 0, 2)).astype(NP_FP8)
        else:
            m["pos4"] = pos4.astype(NP_BF16)
        in_maps.append(m)

    # host-side unshard constants
    bias = (np.asarray(f["pb_b2"], np.float32)
            + np.asarray(f["bv"], np.float32))           # [C]
    return in_maps, bias


_NC_CACHE = {}


def _h1_fp8():
    return os.environ.get("KERNEL_H1BF16", "1") != "1"


def _get_program():
    key = "nc" + ("8" if _h1_fp8() else "16")
    if key not in _NC_CACHE:
        _NC_CACHE[key] = build_program(h1_fp8=_h1_fp8())
    return _NC_CACHE[key]


def unshard(results, bias):
    out = np.zeros((B, N, C), np.float32)
    for core in range(NCORES):
        b, qc = divmod(core, CPB)
        o = np.asarray(results[core]["out"], np.float32)    # [128, T, 2, n]
        es = np.asarray(results[core]["esum"], np.float32)  # [G, NLOC]
        # channel c = j*128 + p ; group g = c // 32 ; point = t*NPT + n
        oc = o.transpose(1, 3, 2, 0).reshape(NLOC, C)       # [n, c]
        denom = np.repeat(es.T, C // G, axis=1) * S_VAL     # [n, c]
        out[b, qc * NLOC:(qc + 1) * NLOC] = oc / denom + bias[None, :]
    return out


def kernel(**inputs):
    nc = _get_program()
    in_maps, bias = host_prep(inputs, h1_fp8=_h1_fp8())
    res = bass_utils.run_bass_kernel_spmd(
        nc, in_maps, list(range(NCORES)),
        trace=bool(int(os.environ.get("KERNEL_TRACE", "0"))))
    _NC_CACHE["last_results"] = res
    return unshard(res.results, bias)

